# revision 25
# baseline (speedup 1.0000x reference)
"""Trainium2 Bass kernel for nn_CrossAttentionBlock (B=2, S=2048, D=1024, H=16, HD=64).

Sharding: 8 cores = 2 batches x 4 head-quads (4 heads each, E=256 channels).
Each core computes q/k/v projections for its quad, RoPE, SDPA, and a partial
output projection [S, D]; host sums the 4 partials per batch and adds the
output bias (biases fold into host-side corrections; zero-bias inputs skip
them entirely).

Device pipeline (matmul operands bf16, fp32 PSUM accumulation):
  - input DMAs are 0.5-1MB k-tile-packed chunks (rearranged 3-D APs) split
    across both HWDGE queues and emitted immediately before their first
    consumer, so the K projection starts ~12us in and chases the DMA stream
    and Tile's DMA-lane waits never alias onto later transfers.
  - K/Q projections accumulate k-OUTER (one 4-bank PSUM slot per pair); the
    RoPE permutation is folded into the weight rows on the host (rot-half
    form: out = q*cos + swap_halves(q)*sin, sign folded into sin), applied
    with a partition-swap DMA plus three DVE multiplies.  V-projection tiles
    are emitted in two blocks that fill the K->Q transition and the q-rope
    tail.
  - SDPA is software-pipelined per (q-chunk, k-tile): scores/exp run one
    k-tile ahead of ctx/den.  Scores use row-tiled concurrent PE pairs
    (contraction 64/head) into five 1-bank PSUM slots; exp is split across
    engines per half-tile: pair 0 -> ACT (true exp), pair 1 -> DVE via the
    Schraudolph bit-trick int16(A*x + B - sigma) bitcast to bf16 (~1% rms
    rel err that diffuses to ~0.2% after the softmax-weighted average).
    ctx^T accumulates via col-tiled concurrent pairs; denominators ride a
    single 4-way col-tiled ones-matmul pass per k-tile (held back with an
    explicit dep so all four tiles pack into one PE pass).
  - per q-chunk: ctx leaves PSUM unnormalized (ACT copies), 1/den runs on a
    DRAM-compacted [128,16] layout (iterative reciprocal at 0.25us), and the
    DRAM roundtrip doubles as the per-head partition broadcast; normalize
    multiplies run on the otherwise-idle GPSIMD.  The last chunk uses a
    latency-optimized variant (direct [128,512] reciprocal, DVE multiplies).
  - the output projection for chunk i interleaves into chunk i+1's SDPA
    (PSUM slots shared with scores), with PSUM->SBUF copies split half/half
    across ACT and DVE so they never head-of-line block a queued exp.
"""
import os
import sys

sys.path.insert(0, "/opt/trn_rl_repo")

import numpy as np
import ml_dtypes

BF16 = ml_dtypes.bfloat16

B, S, D, H = 2, 2048, 1024, 16
HD = D // H          # 64
DIM = HD // 2        # 32
QUADS = 4            # head groups of 4
E = D // QUADS       # 256 channels per core
ROPE_BASE = 10000.0
N_CORES = 8

KTILES = D // 128    # 8
ST = S // 128        # 16 s-tiles
QC = S // 512        # 4 q-chunks
KI = S // 128        # 16 k-tiles per SDPA chunk

# Schraudolph bf16 exp: bits = int16(A*x + (B - sigma)); bitcast to bf16.
EXP_A = 128.0 / float(np.log(2.0))
EXP_B = 127.0 * 128.0
EXP_SIGMA = 2.0


def _host_prep(x_q, x_kv, wq, bq, wk, bk, wv, bv, wo):
    """Build the per-core input maps (all bf16)."""
    perm = np.concatenate([np.arange(0, HD, 2), np.arange(1, HD, 2)])  # even|odd
    scale = 1.0 / np.sqrt(HD)

    freqs = np.exp(-np.arange(DIM, dtype=np.float64) * np.log(ROPE_BASE) / DIM)
    ang = np.arange(S, dtype=np.float64)[:, None] * freqs[None, :]     # [S, 32]
    cos = np.cos(ang).T                                                # [32, S]
    sin = np.sin(ang).T
    cos64 = np.concatenate([cos, cos], axis=0)                         # [64, S]
    sin64 = np.concatenate([-sin, sin], axis=0)
    cosT = np.concatenate([cos64, cos64], axis=0).astype(BF16)         # [128, S]
    sinT = np.concatenate([sin64, sin64], axis=0).astype(BF16)

    def proj_mat(w, permute, s):
        # rows for one quad stacked [256, 1024] -> d-major [1024, 256],
        # then k-tile-packed [128, 8*256] (k-tile k at columns k*256)
        blocks = []
        for h in range(4):
            rows = slice(h * HD, (h + 1) * HD)
            wb = w[rows, :]
            if permute:
                wb = wb[perm, :]
            blocks.append(wb * s)
        wT = np.concatenate(blocks, axis=0).T  # [1024, 256]
        return np.concatenate([wT[k * 128:(k + 1) * 128, :]
                               for k in range(KTILES)], axis=1)  # [128, 2048]

    in_maps = []
    for c in range(N_CORES):
        b_ = c // QUADS
        g = c % QUADS
        hs = slice(g * E, (g + 1) * E)
        im = {
            "xqT": np.ascontiguousarray(x_q[b_].T).astype(BF16),     # [1024, S]
            "xkvT": np.ascontiguousarray(x_kv[b_].T).astype(BF16),
            "wqT": np.ascontiguousarray(
                proj_mat(wq[hs, :], True, scale)).astype(BF16),
            "wkT": np.ascontiguousarray(
                proj_mat(wk[hs, :], True, 1.0)).astype(BF16),
            "wvT": np.ascontiguousarray(
                proj_mat(wv[hs, :], False, 1.0)).astype(BF16),
            "woT": np.ascontiguousarray(wo[:, hs].T).astype(BF16),   # [256, 1024]
            "csT": np.ascontiguousarray(
                np.concatenate([cosT, sinT], axis=1)),               # [128, 2S]
            "ones_col": np.ones((128, 1), dtype=BF16),
        }
        in_maps.append(im)
    return in_maps


# ---------------------------------------------------------------------------
_PROGRAM_CACHE = {}


def _fixed_tile_context(tile_mod, bass_rust_mod, vector_clock_mod):
    """TileContext whose tail drain splits multi-sem waits into single-wait
    NOPs (this walrus rejects >1 sync-wait on one instruction)."""
    SyncInfo = bass_rust_mod.SyncInfo
    ScopedClock = vector_clock_mod.ScopedClock

    class TC(tile_mod.TileContext):
        def _drain_and_barrier(self, tick_clock, wait_clock):
            harvest = self.nc.sync.nop(nofuse=True)
            wait_clock.add_sem_waits(
                harvest.ins, ScopedClock({None: tick_clock.global_clock}))
            si = harvest.ins.sync_info
            waits = list(si.on_wait) if si is not None else []
            if len(waits) > 1:
                harvest.ins.sync_info = SyncInfo(
                    on_wait=[waits[0]], on_update=list(si.on_update))
                for w in waits[1:]:
                    nop = self.nc.sync.nop(nofuse=True)
                    nop.ins.sync_info = SyncInfo(on_wait=[w], on_update=[])
            self.nc.sync.drain()
            self.nc.all_engine_barrier()
            assert self.sems is not None
            popped = self.nc._tile_sem_poison_stack.pop()
            assert popped is self._sem_poison
            self.nc.clear_and_free_semaphores(list(self.sems.allocated().values()))
            self.nc.all_engine_barrier()

    return TC


def _split_multiwait_instructions(nc, mybir, SyncInfo):
    """This walrus build rejects >1 sync-wait per instruction; hoist extra
    waits onto single-wait NOPs inserted just before, on the same engine."""
    ctr = 0
    for blk in nc.m.functions[0].blocks:
        insts = blk.instructions
        i = 0
        while i < len(insts):
            inst = insts[i]
            si = inst.sync_info
            if si is not None and len(si.on_wait) > 1:
                waits = list(si.on_wait)
                inst.sync_info = SyncInfo(on_wait=[waits[-1]],
                                          on_update=list(si.on_update))
                nops = []
                for w in waits[:-1]:
                    nop = mybir.InstNoOp(name=f"waitsplit_{ctr}", ins=[], outs=[])
                    ctr += 1
                    nop.engine = inst.engine
                    nop.sync_info = SyncInfo(on_wait=[w], on_update=[])
                    nops.append(nop)
                insts[i:i] = nops
                i += len(nops)
            i += 1
    return ctr


def build_program(split_waits=True):
    import concourse.bass as bass
    import concourse.mybir as mybir
    import concourse.tile as tile
    import bass_rust
    from concourse import vector_clock
    from concourse.tile import add_dep_helper

    f32 = mybir.dt.float32
    bf16 = mybir.dt.bfloat16
    i16 = mybir.dt.int16
    Exp = mybir.ActivationFunctionType.Exp
    Copy = mybir.ActivationFunctionType.Copy
    mult = mybir.AluOpType.mult
    add = mybir.AluOpType.add

    nc = bass.Bass("TRN2", target_bir_lowering=False, debug=False,
                   num_devices=N_CORES)

    xqT = nc.dram_tensor("xqT", [D, S], bf16, kind="ExternalInput").ap()
    xkvT = nc.dram_tensor("xkvT", [D, S], bf16, kind="ExternalInput").ap()
    wqT = nc.dram_tensor("wqT", [128, KTILES * E], bf16, kind="ExternalInput").ap()
    wkT = nc.dram_tensor("wkT", [128, KTILES * E], bf16, kind="ExternalInput").ap()
    wvT = nc.dram_tensor("wvT", [128, KTILES * E], bf16, kind="ExternalInput").ap()
    woT = nc.dram_tensor("woT", [E, D], bf16, kind="ExternalInput").ap()
    csT = nc.dram_tensor("csT", [128, 2 * S], bf16, kind="ExternalInput").ap()
    ones_col = nc.dram_tensor("ones_col", [128, 1], bf16, kind="ExternalInput").ap()
    out = nc.dram_tensor("out", [S, D], f32, kind="ExternalOutput").ap()

    TC = _fixed_tile_context(tile, bass_rust, vector_clock)

    def kchunk(x, lo, hi):
        # [128, (hi-lo)*S] k-tile-packed view of rows [lo*128, hi*128)
        return x[lo * 128:hi * 128, :].rearrange("(k p) c -> p k c", p=128)

    with TC(nc) as tc:
        with tc.tile_pool(name="persist", bufs=1) as per:
            # ---- input DMAs: 1MB chunks; each DMA is emitted just before
            # its first consumer so Tile's DMA-lane waits never alias onto
            # later transfers, and queue FIFOs prioritize what's needed next.
            xkv_sb = per.tile([128, KTILES * S], bf16, tag="xkv")
            xq_sb = per.tile([128, KTILES * S], bf16, tag="xq")
            nc.sync.dma_start(xkv_sb[:, 0:S], kchunk(xkvT, 0, 1))
            wk_sb = per.tile([128, KTILES * E], bf16, tag="wk")
            nc.scalar.dma_start(wk_sb[:, :], wkT[:, :])
            nc.scalar.dma_start(xkv_sb[:, S:2 * S], kchunk(xkvT, 1, 2))
            nc.sync.dma_start(xkv_sb[:, 2 * S:4 * S], kchunk(xkvT, 2, 4))
            nc.scalar.dma_start(xkv_sb[:, 4 * S:6 * S], kchunk(xkvT, 4, 6))
            nc.sync.dma_start(xkv_sb[:, 6 * S:8 * S], kchunk(xkvT, 6, 8))
            cs_sb = per.tile([128, 2 * S], bf16, tag="cs")
            nc.scalar.dma_start(cs_sb[:, :], csT[:, :])
            cos_sb = cs_sb[:, 0:S]
            sin_sb = cs_sb[:, S:2 * S]
            wq_sb = per.tile([128, KTILES * E], bf16, tag="wq")
            wv_sb = per.tile([128, KTILES * E], bf16, tag="wv")
            ones_sb = per.tile([128, 1], bf16, tag="ones")
            wo_sb = per.tile([128, 2 * D], bf16, tag="wo")   # pair p at cols p*D

            # persistent activations
            qr_sb = [per.tile([128, S], bf16, tag=f"qr{p}", name=f"qr{p}")
                     for p in range(2)]
            kr_sb = [per.tile([128, S], bf16, tag=f"kr{p}", name=f"kr{p}")
                     for p in range(2)]
            v_sb = per.tile([128, ST * E], bf16, tag="v")    # s-tile st at cols st*E
            ctxu_sb = [per.tile([128, S], bf16, tag=f"ctxu{p}", name=f"ctxu{p}")
                       for p in range(2)]
            ctxn_sb = [per.tile([128, S], bf16, tag=f"ctxn{p}", name=f"ctxn{p}")
                       for p in range(2)]

            # ---- projections: K pair0/pair1, Q pair0/pair1 (one 4-bank PSUM
            # slot, k-OUTER accumulation chasing the DMA chunks), then V
            # (1-bank slot; V matmuls also fill Q's input stalls).
            with tc.tile_pool(name="qk_ps", bufs=1, space="PSUM") as pps, \
                 tc.tile_pool(name="qk_tmp", bufs=2) as tmp, \
                 tc.tile_pool(name="v_ps", bufs=2, space="PSUM") as vps:

                def qk_pair(w_sb_, x_sb_, dst, p, rope_chunk, swap_q):
                    q_ps = pps.tile([128, S], f32, tag="qp")
                    for k in range(KTILES):
                        for sc in range(QC):
                            ss = slice(sc * 512, (sc + 1) * 512)
                            nc.tensor.matmul(
                                q_ps[:, ss],
                                lhsT=w_sb_[:, k * E + p * 128:
                                           k * E + (p + 1) * 128],
                                rhs=x_sb_[:, k * S + sc * 512:
                                          k * S + (sc + 1) * 512],
                                start=(k == 0), stop=(k == KTILES - 1))
                    for cc in range(0, S, rope_chunk):
                        cs_ = slice(cc, cc + rope_chunk)
                        qb = tmp.tile([128, rope_chunk], bf16, tag="qb")
                        qsw = tmp.tile([128, rope_chunk], bf16, tag="qsw")
                        qcos = tmp.tile([128, rope_chunk], bf16, tag="qcos")
                        nc.scalar.activation(qb[:, :], q_ps[:, cs_], Copy)
                        for a, bdst in ((0, 32), (32, 0), (64, 96), (96, 64)):
                            nc.scalar.dma_start(qsw[bdst:bdst + 32, :],
                                                qb[a:a + 32, :])
                        nc.vector.tensor_tensor(qcos[:, :], qb[:, :],
                                                cos_sb[:, cs_], mult)
                        nc.vector.tensor_tensor(qsw[:, :], qsw[:, :],
                                                sin_sb[:, cs_], mult)
                        nc.vector.tensor_tensor(dst[:, cs_], qcos[:, :],
                                                qsw[:, :], add)

                def v_tile(st):
                    v_ps = vps.tile([128, E], f32, tag="v_ps")
                    for k in range(KTILES):
                        nc.tensor.matmul(
                            v_ps[:, :],
                            lhsT=xkv_sb[:, k * S + st * 128:
                                        k * S + (st + 1) * 128],
                            rhs=wv_sb[:, k * E:(k + 1) * E],
                            start=(k == 0), stop=(k == KTILES - 1))
                    nc.vector.tensor_copy(v_sb[:, st * E:(st + 1) * E],
                                          v_ps[:, :])

                qk_pair(wk_sb, xkv_sb, kr_sb[0], 0, S, True)
                # later inputs are emitted near their consumers so Tile's
                # DMA-lane waits for earlier consumers never alias onto them
                nc.scalar.dma_start(wv_sb[:, :], wvT[:, :])
                nc.sync.dma_start(wq_sb[:, :], wqT[:, :])
                for c in range(4):
                    nc.sync.dma_start(xq_sb[:, c * 2 * S:(c + 1) * 2 * S],
                                      kchunk(xqT, 2 * c, 2 * c + 2))
                qk_pair(wk_sb, xkv_sb, kr_sb[1], 1, S, True)
                # V tiles 0-7 fill the K->Q transition and any xq-DMA stalls
                for st in range(8):
                    v_tile(st)
                qk_pair(wq_sb, xq_sb, qr_sb[0], 0, S, False)
                qk_pair(wq_sb, xq_sb, qr_sb[1], 1, S, False)
                nc.sync.dma_start(ones_sb[:, :], ones_col[:, :])
                for p in range(2):
                    nc.scalar.dma_start(wo_sb[:, p * D:(p + 1) * D],
                                        woT[p * 128:(p + 1) * 128, :])
                # V tiles 8-15 fill the q-rope tail before SDPA
                for st in range(8, ST):
                    v_tile(st)

            # ---- SDPA, software-pipelined: scores/exp run one k-tile ahead
            # of ctx/den so the exp latency never gates the PE. ----
            with tc.tile_pool(name="sc_ps", bufs=5, space="PSUM") as scp, \
                 tc.tile_pool(name="ctx_ps", bufs=1, space="PSUM") as cdp, \
                 tc.tile_pool(name="den_ps", bufs=1, space="PSUM") as dnp, \
                 tc.tile_pool(name="e_sb", bufs=4) as esp, \
                 tc.tile_pool(name="norm", bufs=2) as nrm, \
                 tc.tile_pool(name="o_sb", bufs=3) as osb, \
                 tc.tile_pool(name="ldram", bufs=2, space="DRAM") as ldr:

                def scores_exp(qh, ki):
                    # each head-half gets its own 1-bank PSUM slot and its
                    # own half-tile exp, so slot recycling waits ~0.65us
                    # (one half-exp) instead of a full-tile exp
                    qs = slice(qh * 512, (qh + 1) * 512)
                    ks = slice(ki * 128, (ki + 1) * 128)
                    e_pair = []
                    for p in range(2):
                        e_sb = esp.tile([128, 1024], bf16, tag="e")
                        last_exp = None
                        for half in range(2):
                            s_ps = scp.tile([128, 512], f32, tag="s")
                            nc.tensor.matmul(
                                s_ps[:, :],
                                lhsT=kr_sb[p][half * 64:(half + 1) * 64, ks],
                                rhs=qr_sb[p][half * 64:(half + 1) * 64, qs],
                                tile_position=(half * 64, 0),
                                start=True, stop=True)
                            eh = e_sb[:, half * 512:(half + 1) * 512]
                            if p == 0:
                                last_exp = nc.scalar.activation(
                                    eh, s_ps[:, :], Exp)
                            else:
                                last_exp = nc.vector.tensor_scalar(
                                    eh.bitcast(i16), s_ps[:, :],
                                    float(EXP_A), float(EXP_B - EXP_SIGMA),
                                    mult, add)
                        e_pair.append((e_sb, last_exp))
                    return e_pair

                def ctx_den(ctx_ps, den_ps, ki, e_pair):
                    for p in range(2):
                        e_sb = e_pair[p][0]
                        nc.tensor.matmul(
                            ctx_ps[p][0:64, :],
                            lhsT=v_sb[:, ki * E + (2 * p) * 64:
                                      ki * E + (2 * p) * 64 + 64],
                            rhs=e_sb[:, 0:512],
                            tile_position=(0, 0),
                            start=(ki == 0), stop=(ki == KI - 1),
                            skip_group_check=True)
                        nc.tensor.matmul(
                            ctx_ps[p][64:128, :],
                            lhsT=v_sb[:, ki * E + (2 * p + 1) * 64:
                                      ki * E + (2 * p + 1) * 64 + 64],
                            rhs=e_sb[:, 512:1024],
                            tile_position=(0, 64),
                            start=(ki == 0), stop=(ki == KI - 1),
                            skip_group_check=True)
                    for gidx, (p, half) in enumerate(
                            ((0, 0), (0, 1), (1, 0), (1, 1))):
                        den_mm = nc.tensor.matmul(
                            den_ps[gidx * 32: gidx * 32 + 1, :],
                            lhsT=ones_sb[:, :],
                            rhs=e_pair[p][0][:, half * 512:(half + 1) * 512],
                            tile_position=(0, gidx * 32),
                            start=(ki == 0), stop=(ki == KI - 1),
                            skip_group_check=True)
                        if p == 0:
                            add_dep_helper(den_mm.ins, e_pair[1][1].ins,
                                           sync=False, reason="pack den 4-way")

                def o_proj_group(st, ch, on_dve=False):
                    o_ps = scp.tile([128, 512], f32, tag="s")
                    for p in range(2):
                        nc.tensor.matmul(
                            o_ps[:, :],
                            lhsT=ctxn_sb[p][:, st * 128:(st + 1) * 128],
                            rhs=wo_sb[:, p * D + ch * 512:
                                      p * D + (ch + 1) * 512],
                            start=(p == 0), stop=(p == 1))
                    o_out = osb.tile([128, 512], f32, tag="oo")
                    # half on each engine: small quanta so the copy never
                    # head-of-line blocks a queued exp for long
                    nc.scalar.activation(o_out[:, 0:256], o_ps[:, 0:256],
                                         Copy)
                    nc.vector.tensor_copy(o_out[:, 256:512], o_ps[:, 256:512])
                    nc.scalar.dma_start(
                        out[st * 128:(st + 1) * 128,
                            ch * 512:(ch + 1) * 512], o_out[:, :])

                def normalize(qh, den_ps, fast_tail=False):
                    # 1/den on a compacted [128,16] layout; the DRAM roundtrip
                    # doubles as the partition broadcast.  The last chunk uses
                    # a latency-optimized variant: direct [128,512] reciprocal
                    # on the (then idle) DVE skips two DRAM hops, broadcasts
                    # are split per pair, and the normalize multiplies run on
                    # the DVE so the tail o-projection starts sooner.
                    qs = slice(qh * 512, (qh + 1) * 512)
                    den_sb = nrm.tile([128, 512], f32, tag="densb")
                    nc.scalar.activation(den_sb[:, :], den_ps[:, :], Copy)
                    dlin = ldr.tile([4, 512], bf16, tag="dlin")
                    if fast_tail:
                        linv5 = nrm.tile([128, 512], bf16, tag="linv5")
                        with nc.allow_low_precision(reason="bf16 1/den"):
                            nc.vector.reciprocal(linv5[:, :], den_sb[:, :])
                        for p in range(2):
                            nc.sync.dma_start(
                                dlin[2 * p:2 * p + 2, :],
                                linv5[p * 64:p * 64 + 64:32, :])
                    else:
                        dscr = ldr.tile([4, 512], f32, tag="dscr")
                        nc.sync.dma_start(dscr[:, :], den_sb[0:128:32, :])
                        dcmp = nrm.tile([128, 16], f32, tag="dcmp")
                        nc.sync.dma_start(dcmp[:, :], dscr[:, :])
                        linv = nrm.tile([128, 16], bf16, tag="linv")
                        with nc.allow_low_precision(reason="bf16 1/den"):
                            nc.vector.reciprocal(linv[:, :], dcmp[:, :])
                        nc.sync.dma_start(dlin[:, :], linv[:, :])
                    lbc = [nrm.tile([128, 512], bf16, tag=f"lbc{p}",
                                    name=f"lbc{p}") for p in range(2)]
                    for gidx, (p, half) in enumerate(
                            ((0, 0), (0, 1), (1, 0), (1, 1))):
                        nc.sync.dma_start(
                            lbc[p][half * 64:(half + 1) * 64, :],
                            dlin[gidx:gidx + 1, :].partition_broadcast(64))
                    return lbc

                def normalize_tts(qh, lbc, fast_tail=False):
                    qs = slice(qh * 512, (qh + 1) * 512)
                    for p in range(2):
                        if fast_tail:
                            nc.vector.tensor_tensor(
                                ctxn_sb[p][:, qs], ctxu_sb[p][:, qs],
                                lbc[p][:, :], mult)
                        else:
                            nc.gpsimd.tensor_tensor(
                                ctxn_sb[p][:, qs], ctxu_sb[p][:, qs],
                                lbc[p][:, :], mult)

                pairs = [(qh, ki) for qh in range(QC) for ki in range(KI)]
                ctx_ps = den_ps = None
                e_cur = scores_exp(*pairs[0])
                for idx, (qh, ki) in enumerate(pairs):
                    qs = slice(qh * 512, (qh + 1) * 512)
                    if ki == 0:
                        ctx_ps = [cdp.tile([128, 512], f32, tag=f"ctx{p}",
                                           name=f"ctx{p}") for p in range(2)]
                        den_ps = dnp.tile([128, 512], f32, tag="den")
                        # ACT "memset": 0*x + 1.0 (keeps the DVE queue
                        # exp-only; kr is just a resident dummy input)
                        nc.scalar.activation(den_ps[:, :],
                                             kr_sb[0][:, 0:512], Copy,
                                             scale=0.0, bias=1.0)
                    # interleaved o-proj of the previous chunk, emitted
                    # before the next scores/exp so its PSUM->SBUF copy sits
                    # ahead of the exps in the engine FIFOs; scheduled late in
                    # the k loop so the final groups execute during this
                    # chunk's normalize latency
                    if qh >= 1 and ki >= 8:
                        g = ki - 8
                        o_proj_group((qh - 1) * 4 + g // 2, g % 2)
                    e_next = (scores_exp(*pairs[idx + 1])
                              if idx + 1 < len(pairs) else None)
                    ctx_den(ctx_ps, den_ps, ki, e_cur)
                    e_cur = e_next
                    if ki == KI - 1:
                        # den/reciprocal/broadcast chain first (its den copy
                        # heads the ACT queue), ctxu copies next, then the
                        # normalize multiplies
                        ft = qh == QC - 1
                        lbc = normalize(qh, den_ps, fast_tail=ft)
                        nc.scalar.activation(ctxu_sb[0][:, qs],
                                             ctx_ps[0][:, :], Copy)
                        nc.scalar.activation(ctxu_sb[1][:, qs],
                                             ctx_ps[1][:, :], Copy)
                        normalize_tts(qh, lbc, fast_tail=ft)
                # tail: o-proj for the last chunk (scp slots give a
                # multi-buffer pipeline; copies split across both engines)
                for st in range(12, 16):
                    for ch in range(2):
                        o_proj_group(st, ch)

    if split_waits:
        _split_multiwait_instructions(nc, mybir, bass_rust.SyncInfo)
    return nc


def kernel(x_q, x_kv, wq, bq, wk, bk, wv, bv, wo, bo):
    from concourse import bass_utils

    x_q = np.asarray(x_q, dtype=np.float32)
    x_kv = np.asarray(x_kv, dtype=np.float32)
    wq = np.asarray(wq, dtype=np.float32); bq = np.asarray(bq, dtype=np.float32)
    wk = np.asarray(wk, dtype=np.float32); bk = np.asarray(bk, dtype=np.float32)
    wv = np.asarray(wv, dtype=np.float32); bv = np.asarray(bv, dtype=np.float32)
    wo = np.asarray(wo, dtype=np.float32); bo = np.asarray(bo, dtype=np.float32)

    out_bias = bo.copy()
    if np.any(bv):
        # ctx = attn @ (v + bv) = attn @ v + bv (attn rows sum to 1), and bv
        # maps through wo onto the host-side output bias.
        out_bias += wo @ bv
    if np.any(bq) or np.any(bk):
        # q/k biases rotate with RoPE, so fold them into the inputs instead:
        # (x + c) @ w.T == x @ w.T + b  when  w @ c == b  (w square,
        # generically invertible).  The k-fold also perturbs v by wv @ ck,
        # which maps through wo and is subtracted on the host.
        cq = np.linalg.solve(wq.astype(np.float64),
                             bq.astype(np.float64)).astype(np.float32)
        ck = np.linalg.solve(wk.astype(np.float64),
                             bk.astype(np.float64)).astype(np.float32)
        x_q = x_q + cq[None, None, :]
        x_kv = x_kv + ck[None, None, :]
        out_bias -= wo @ (wv @ ck)

    in_maps = _host_prep(x_q, x_kv, wq, bq, wk, bk, wv, bv, wo)

    if "prog" not in _PROGRAM_CACHE:
        _PROGRAM_CACHE["prog"] = build_program()
    nc = _PROGRAM_CACHE["prog"]

    res = bass_utils.run_bass_kernel_spmd(
        nc, in_maps, core_ids=list(range(N_CORES)),
        trace=os.environ.get("KERNEL_TRACE", "") == "1")
    _PROGRAM_CACHE["last_result"] = res

    out = np.zeros((B, S, D), dtype=np.float32)
    for c in range(N_CORES):
        out[c // QUADS] += res.results[c]["out"]
    out += out_bias[None, None, :]
    return out
